# revision 1
# baseline (speedup 1.0000x reference)
"""HAN (heterogeneous graph attention) kernel for 8 Trainium2 NeuronCores.

Strategy (dst-partitioned, gather + one-hot-matmul aggregation):
  - Each core owns dst rows [c*12500, (c+1)*12500) of the mat nodes.
  - P1: every core redundantly projects the full node tables into DRAM:
      T1  [100118, 136] = [h | aS]  (mat rows 0:100000 w/ a_src_mm folded,
                                     elem rows 100000:100118 w/ a_src_em)
      ADt [100000, 16]  = [aD_em | aD_mm]
    The per-head <h, a_src>/<h, a_dst> reductions are folded into the
    projection weights on the host (block-diagonal compose), so one matmul
    per 128-node tile emits h, aS and both aD tables at once.
  - P2: edges are host-bucketed by (core, dst_tile-of-128) and padded to a
    fixed number of 128-edge chunks per tile (em chunks then mm chunks).
    Per tile: one multi-row indirect DMA gathers [h|aS] rows by src, a second
    gathers aD rows by (2*dst+tag); alpha -> leakyrelu -> exp on-chip; the
    exp overwrite lands in the gathered rows' aS slot so each chunk's
    [weighted-h | ex] is matmul-ready; a one-hot (iota == dst_local) matrix
    per chunk scatter-adds into PSUM [128 dst, 136] (numerator | denominator).
    Softmax max-subtraction is skipped (alphas are O(1); exp is safe in fp32
    and softmax is shift-invariant) so a single pass suffices.
  - Per-tile: o = relu(num/(den+1e-16)), PE-transposed and stored as
    oT [128 feat, 12544] per metapath; tanh/semantic partial sums accumulate.
  - P3: 1KB AllReduce of the semantic partials, softmax over the 2 metapaths,
    weighted combine + final linear, output [12544, 64] rows per core.
"""

import numpy as np

import concourse.bacc as bacc
import concourse.bass as bass
import concourse.mybir as mybir
import concourse.tile as tile
from concourse.bass_utils import run_bass_kernel_spmd
from concourse.masks import make_identity

P = 128
N_MAT = 100000
N_ELEM = 118
F_MAT = 128
F_ELEM = 64
HID = 128
H = 8
D = 16
OUT = 64
NCORES = 8
ND = N_MAT // NCORES          # 12500 dst rows per core
NT = (ND + P - 1) // P        # 98 dst tiles per core (last has 84 rows)
NDP = NT * P                  # 12544 padded dst rows
NEG = 0.2
NTM_FULL = N_MAT // 1024      # 97 full 1024-row projection slabs
LAST_SLAB = N_MAT - NTM_FULL * 1024  # 672 = 5*128 + 32
F32 = mybir.dt.float32
I32 = mybir.dt.int32


def _blockdiag(a):
    """a [H, D] -> [HID, H] block diagonal so h @ A = per-head <h, a>."""
    A = np.zeros((HID, H), np.float32)
    for h in range(H):
        A[h * D:(h + 1) * D, h] = a[h]
    return A


def build_host_tensors(inputs):
    """All host-side preprocessing that is independent of edge bucketing."""
    x_mat = inputs["x_mat"]
    WpT = np.ascontiguousarray(inputs["W_proj_mat"].T)       # [128f, 128k]
    WpeT = np.ascontiguousarray(inputs["W_proj_elem"].T)     # [64f, 128k]
    b_m = inputs["b_proj_mat"]
    b_e = inputs["b_proj_elem"]
    A_smm = _blockdiag(inputs["a_src_mm"])
    A_dem = _blockdiag(inputs["a_dst_em"])
    A_dmm = _blockdiag(inputs["a_dst_mm"])
    A_sem = _blockdiag(inputs["a_src_em"])

    wmat = np.concatenate(
        [WpT, WpT @ A_smm, WpT @ A_dem, WpT @ A_dmm], axis=1
    ).astype(np.float32)                                     # [128, 152]
    bmat_row = np.concatenate([b_m, b_m @ A_smm, b_m @ A_dem, b_m @ A_dmm])
    bmat = np.tile(bmat_row.astype(np.float32), (P, 1))      # [128, 152]
    wel = np.concatenate([WpeT, WpeT @ A_sem], axis=1).astype(np.float32)
    bel_row = np.concatenate([b_e, b_e @ A_sem])
    bel = np.tile(bel_row.astype(np.float32), (P, 1))        # [128, 136]

    host = dict(
        xT=np.ascontiguousarray(x_mat.T).astype(np.float32),
        xeT=np.ascontiguousarray(inputs["x_elem"].T).astype(np.float32),
        wmat=wmat,
        bmat=bmat,
        wel=wel,
        bel=bel,
        wkT=np.ascontiguousarray(inputs["Wk"].T).astype(np.float32),
        bkc=inputs["bk"].astype(np.float32)[:, None],
        qc=(inputs["q"] / float(N_MAT)).astype(np.float32)[:, None],
        wlT=np.ascontiguousarray(inputs["Wl"].T).astype(np.float32),
        blb=np.tile(inputs["bl"].astype(np.float32), (P, 1)),
        iot=np.tile(np.arange(P, dtype=np.float32), (P, 1)),
        ones=np.ones((1, P), np.float32),
    )
    return host


def bucket_edges(src, dst, tag):
    """Bucket edges by (core, dst_tile). Returns per-core arrays
    [NCORES, NT, P, CH] of (src_row_idx, ad_row_idx, dst_local_f32)."""
    core = dst // ND
    rem = dst % ND
    tl = rem // P
    dl = rem % P
    key = core * NT + tl
    order = np.argsort(key, kind="stable")
    counts = np.bincount(key, minlength=NCORES * NT)
    CH = int((counts.max() + P - 1) // P)
    starts = np.zeros(NCORES * NT, np.int64)
    starts[1:] = np.cumsum(counts)[:-1]
    ks = key[order]
    rank = np.arange(len(order)) - starts[ks]
    lane = rank % P
    chunk = rank // P
    flat = (ks * P + lane) * CH + chunk

    if tag == 0:  # em: src indexes elem rows of T1; ad row = 2*dst
        srcv = (N_MAT + src).astype(np.int32)
        adv = (2 * dst).astype(np.int32)
    else:         # mm
        srcv = src.astype(np.int32)
        adv = (2 * dst + 1).astype(np.int32)

    a_src = np.zeros(NCORES * NT * P * CH, np.int32)
    a_ad = np.zeros(NCORES * NT * P * CH, np.int32)
    a_dl = np.full(NCORES * NT * P * CH, 300.0, np.float32)
    a_src[flat] = srcv[order]
    a_ad[flat] = adv[order]
    a_dl[flat] = dl[order].astype(np.float32)
    shp = (NCORES, NT, P, CH)
    return a_src.reshape(shp), a_ad.reshape(shp), a_dl.reshape(shp), CH


def build_program(ch_em, ch_mm, dbg=False):
    CH = ch_em + ch_mm
    nc = bacc.Bacc(
        "TRN2",
        target_bir_lowering=False,
        debug=False,
        enable_asserts=False,
        num_devices=NCORES,
    )

    inp = {}
    def din(name, shape, dt=F32):
        inp[name] = nc.dram_tensor(name, list(shape), dt, kind="ExternalInput").ap()
        return inp[name]

    xT = din("xT", [F_MAT, N_MAT])
    xeT = din("xeT", [F_ELEM, N_ELEM])
    wmat = din("wmat", [F_MAT, 152])
    bmat = din("bmat", [P, 152])
    wel = din("wel", [F_ELEM, 136])
    bel = din("bel", [P, 136])
    esrc = din("esrc", [NT, P, CH], I32)
    ead = din("ead", [NT, P, CH], I32)
    edl = din("edl", [NT, P, CH])
    wkT = din("wkT", [HID, HID])
    bkc = din("bkc", [HID, 1])
    qc = din("qc", [HID, 1])
    wlT = din("wlT", [HID, OUT])
    blb = din("blb", [P, OUT])
    iot = din("iot", [P, P])
    ones = din("ones", [1, P])
    y = nc.dram_tensor("y", [NDP, OUT], F32, kind="ExternalOutput").ap()

    with tile.TileContext(nc) as tc:
        with (
            tc.tile_pool(name="const", bufs=1) as cp,
            tc.tile_pool(name="dram", bufs=1, space="DRAM") as dp,
        ):
            # ---- persistent DRAM tables ----
            if dbg:
                kind = "ExternalOutput"
                T1 = nc.dram_tensor("dT1", [N_MAT + N_ELEM, 136], F32, kind=kind).ap()
                ADt = nc.dram_tensor("dADt", [N_MAT, 16], F32, kind=kind).ap()
                oemT = nc.dram_tensor("doemT", [HID, NDP], F32, kind=kind).ap()
                ommT = nc.dram_tensor("dommT", [HID, NDP], F32, kind=kind).ap()
                Gd = nc.dram_tensor("dG", [P, CH, 136], F32, kind=kind).ap()
                OHd = nc.dram_tensor("dOH", [P, CH, 128], F32, kind=kind).ap()
            else:
                T1 = dp.tile([N_MAT + N_ELEM, 136], F32)
                ADt = dp.tile([N_MAT, 16], F32)
                oemT = dp.tile([HID, NDP], F32)
                ommT = dp.tile([HID, NDP], F32)
            Sin_d = dp.tile([HID, 2], F32)
            Sout_d = dp.tile([HID, 2], F32)

            # ---- constants in SBUF ----
            def lc(ap_in, shape, tag):
                t = cp.tile(list(shape), F32, tag=tag)
                nc.sync.dma_start(out=t[:], in_=ap_in[:])
                return t

            wmat_sb = lc(wmat, [F_MAT, 152], "wmat")
            bmat_sb = lc(bmat, [P, 152], "bmat")
            wel_sb = lc(wel, [F_ELEM, 136], "wel")
            bel_sb = lc(bel, [P, 136], "bel")
            wkT_sb = lc(wkT, [HID, HID], "wkT")
            bkc_sb = lc(bkc, [HID, 1], "bkc")
            qc_sb = lc(qc, [HID, 1], "qc")
            wlT_sb = lc(wlT, [HID, OUT], "wlT")
            blb_sb = lc(blb, [P, OUT], "blb")
            iot_sb = lc(iot, [P, P], "iot")
            ones_sb = lc(ones, [1, P], "ones")
            ident = cp.tile([P, P], F32, tag="ident")
            make_identity(nc, ident[:])
            S_sb = cp.tile([HID, 2], F32, tag="S")
            nc.gpsimd.memset(S_sb[:], 0.0)

            # ================= P1: projection =================
            with (
                tc.tile_pool(name="p1s", bufs=3) as p1s,
                tc.tile_pool(name="p1p", bufs=2, space="PSUM") as p1p,
            ):
                for s in range(NTM_FULL + 1):
                    w = 1024 if s < NTM_FULL else LAST_SLAB
                    xsl = p1s.tile([P, 1024], F32, tag="xsl")
                    nc.sync.dma_start(
                        out=xsl[:, 0:w], in_=xT[:, s * 1024: s * 1024 + w]
                    )
                    ev = p1s.tile([P, 8, 152], F32, tag="ev")
                    ntile = (w + P - 1) // P
                    for j in range(ntile):
                        m = min(P, w - j * P)
                        ps = p1p.tile([P, 152], F32, tag="ps")
                        nc.tensor.matmul(
                            out=ps[0:m, :],
                            lhsT=xsl[:, j * P: j * P + m],
                            rhs=wmat_sb[:],
                            start=True,
                            stop=True,
                        )
                        nc.vector.tensor_add(
                            out=ev[0:m, j, :], in0=ps[0:m, :], in1=bmat_sb[0:m, :]
                        )
                    rows = s * 1024
                    if s < NTM_FULL:
                        nc.sync.dma_start(
                            out=T1[rows: rows + 1024, :].rearrange(
                                "(a p) e -> p a e", p=P
                            ),
                            in_=ev[:, :, 0:136],
                        )
                        nc.sync.dma_start(
                            out=ADt[rows: rows + 1024, :].rearrange(
                                "(a p) e -> p a e", p=P
                            ),
                            in_=ev[:, :, 136:152],
                        )
                    else:
                        for j in range(ntile):
                            m = min(P, w - j * P)
                            r0 = rows + j * P
                            nc.sync.dma_start(
                                out=T1[r0: r0 + m, :], in_=ev[0:m, j, 0:136]
                            )
                            nc.sync.dma_start(
                                out=ADt[r0: r0 + m, :], in_=ev[0:m, j, 136:152]
                            )
                # elem projection
                xe_sb = p1s.tile([F_ELEM, N_ELEM], F32, tag="xe")
                nc.sync.dma_start(out=xe_sb[:], in_=xeT[:])
                pse = p1p.tile([P, 152], F32, tag="ps")
                nc.tensor.matmul(
                    out=pse[0:N_ELEM, 0:136],
                    lhsT=xe_sb[:],
                    rhs=wel_sb[:],
                    start=True,
                    stop=True,
                )
                eve = p1s.tile([P, 8, 152], F32, tag="ev")
                nc.vector.tensor_add(
                    out=eve[0:N_ELEM, 0, 0:136],
                    in0=pse[0:N_ELEM, 0:136],
                    in1=bel_sb[0:N_ELEM, :],
                )
                nc.sync.dma_start(
                    out=T1[N_MAT: N_MAT + N_ELEM, :], in_=eve[0:N_ELEM, 0, 0:136]
                )

            # ================= P2: edge aggregation =================
            ADt2 = ADt[:].rearrange("n (a b) -> (n a) b", a=2)
            with (
                tc.tile_pool(name="p2s", bufs=2) as p2s,
                tc.tile_pool(name="p2p", bufs=2, space="PSUM") as p2p,
                tc.tile_pool(name="p2t", bufs=2, space="PSUM") as p2t,
                tc.tile_pool(name="p2k", bufs=2, space="PSUM") as p2k,
            ):
                for t in range(NT):
                    src_sb = p2s.tile([P, CH], I32, tag="src")
                    ad_sb = p2s.tile([P, CH], I32, tag="ad")
                    dl_sb = p2s.tile([P, CH], F32, tag="dl")
                    nc.sync.dma_start(out=src_sb[:], in_=esrc[t, :, :])
                    nc.sync.dma_start(out=ad_sb[:], in_=ead[t, :, :])
                    nc.sync.dma_start(out=dl_sb[:], in_=edl[t, :, :])
                    G = p2s.tile([P, CH, 136], F32, tag="G")
                    AD = p2s.tile([P, CH, 8], F32, tag="AD")
                    OH = p2s.tile([P, CH, 128], F32, tag="OH")
                    # one gather per 128-edge chunk: idx [128,1] is the only
                    # index shape the indirect-DMA ucode handles correctly
                    for c in range(CH):
                        nc.gpsimd.indirect_dma_start(
                            out=G[:, c, :],
                            out_offset=None,
                            in_=T1[:, :],
                            in_offset=bass.IndirectOffsetOnAxis(
                                ap=src_sb[:, c:c + 1], axis=0),
                        )
                        nc.gpsimd.indirect_dma_start(
                            out=AD[:, c, :],
                            out_offset=None,
                            in_=ADt2,
                            in_offset=bass.IndirectOffsetOnAxis(
                                ap=ad_sb[:, c:c + 1], axis=0),
                        )
                    if dbg and t == 0:
                        nc.sync.dma_start(out=Gd[:], in_=G[:, :, :])
                    # alpha = aS[src] + aD[dst] ; leakyrelu ; exp -> G aS slot
                    nc.vector.tensor_add(
                        out=AD[:, :, :], in0=AD[:, :, :], in1=G[:, :, 128:136]
                    )
                    nc.vector.scalar_tensor_tensor(
                        out=AD[:, :, :],
                        in0=AD[:, :, :],
                        scalar=NEG,
                        in1=AD[:, :, :],
                        op0=mybir.AluOpType.mult,
                        op1=mybir.AluOpType.max,
                    )
                    nc.scalar.activation(
                        out=G[:, :, 128:136],
                        in_=AD[:, :, :],
                        func=mybir.ActivationFunctionType.Exp,
                    )
                    # weight the gathered rows by ex (broadcast 16-wide)
                    nc.vector.tensor_mul(
                        out=G[:, :, 0:128].rearrange("p c (e s) -> p c e s", s=16),
                        in0=G[:, :, 0:128].rearrange("p c (e s) -> p c e s", s=16),
                        in1=G[:, :, 128:136, None].to_broadcast([P, CH, 8, 16]),
                    )
                    # one-hot: OH[p, c, d] = (iota[d] == dl[p, c])
                    nc.vector.tensor_tensor(
                        out=OH[:, :, :],
                        in0=iot_sb[:, None, :].to_broadcast([P, CH, P]),
                        in1=dl_sb[:, :, None].to_broadcast([P, CH, P]),
                        op=mybir.AluOpType.is_equal,
                    )
                    if dbg and t == 0:
                        nc.sync.dma_start(out=OHd[:], in_=OH[:, :, :])
                    pem = p2p.tile([P, 136], F32, tag="em")
                    pmm = p2p.tile([P, 136], F32, tag="mm")
                    for c in range(CH):
                        tgt = pem if c < ch_em else pmm
                        nc.tensor.matmul(
                            out=tgt[:],
                            lhsT=OH[:, c, :],
                            rhs=G[:, c, :],
                            start=(c == 0 or c == ch_em),
                            stop=(c == ch_em - 1 or c == CH - 1),
                        )
                    for mp, ps_, oTd in ((0, pem, oemT), (1, pmm, ommT)):
                        den = p2s.tile([P, 8], F32, tag=f"den{mp}")
                        nc.vector.tensor_scalar_add(
                            out=den[:], in0=ps_[:, 128:136], scalar1=1e-16
                        )
                        nc.vector.reciprocal(out=den[:], in_=den[:])
                        o_sb = p2s.tile([P, 128], F32, tag=f"o{mp}")
                        nc.vector.tensor_mul(
                            out=o_sb[:].rearrange("p (e s) -> p e s", s=16),
                            in0=ps_[:, 0:128].rearrange("p (e s) -> p e s", s=16),
                            in1=den[:, :, None].to_broadcast([P, 8, 16]),
                        )
                        ptr = p2t.tile([P, P], F32, tag="tr")
                        nc.tensor.transpose(
                            out=ptr[:], in_=o_sb[:], identity=ident[:]
                        )
                        oT_sb = p2s.tile([P, P], F32, tag=f"oT{mp}")
                        nc.scalar.activation(
                            out=oT_sb[:],
                            in_=ptr[:],
                            func=mybir.ActivationFunctionType.Relu,
                        )
                        nc.sync.dma_start(
                            out=oTd[:, t * P:(t + 1) * P], in_=oT_sb[:]
                        )
                        # exclude the 44 pad dst rows of the last tile from
                        # the semantic mean (reference averages over exactly
                        # N_MAT nodes)
                        nw = ND - (NT - 1) * P if t == NT - 1 else P
                        pk = p2k.tile([P, P], F32, tag="k")
                        nc.tensor.matmul(
                            out=pk[:, 0:nw], lhsT=wkT_sb[:], rhs=oT_sb[:, 0:nw],
                            start=True, stop=True,
                        )
                        tanh_sb = p2s.tile([P, P], F32, tag="tanh")
                        s_col = p2s.tile([P, 1], F32, tag="scol")
                        nc.scalar.activation(
                            out=tanh_sb[:, 0:nw],
                            in_=pk[:, 0:nw],
                            func=mybir.ActivationFunctionType.Tanh,
                            bias=bkc_sb[:, 0:1],
                            accum_out=s_col[:],
                        )
                        nc.vector.tensor_add(
                            out=S_sb[:, mp:mp + 1],
                            in0=S_sb[:, mp:mp + 1],
                            in1=s_col[:],
                        )

            # ================= P3: semantic attention + final =================
            with (
                tc.tile_pool(name="p3s", bufs=3) as p3s,
                tc.tile_pool(name="p3p", bufs=2, space="PSUM") as p3p,
            ):
                nc.sync.dma_start(out=Sin_d[:], in_=S_sb[:])
                nc.gpsimd.collective_compute(
                    "AllReduce",
                    mybir.AluOpType.add,
                    replica_groups=[list(range(NCORES))],
                    ins=[Sin_d.opt()],
                    outs=[Sout_d.opt()],
                )
                Sr_sb = p3s.tile([HID, 2], F32, tag="Sr")
                nc.sync.dma_start(out=Sr_sb[:], in_=Sout_d[:])
                ps_s = p3p.tile([P, 2], F32, tag="s")
                nc.tensor.matmul(
                    out=ps_s[0:1, :], lhsT=qc_sb[:, 0:1], rhs=Sr_sb[:],
                    start=True, stop=True,
                )
                es = p3s.tile([P, 2], F32, tag="es")
                nc.scalar.activation(
                    out=es[0:1, :], in_=ps_s[0:1, :],
                    func=mybir.ActivationFunctionType.Exp,
                )
                ds = p3s.tile([P, 1], F32, tag="ds")
                nc.vector.tensor_reduce(
                    out=ds[0:1, :], in_=es[0:1, :],
                    axis=mybir.AxisListType.X, op=mybir.AluOpType.add,
                )
                nc.vector.reciprocal(out=ds[0:1, :], in_=ds[0:1, :])
                at = p3s.tile([P, 2], F32, tag="at")
                nc.vector.tensor_scalar_mul(
                    out=at[0:1, :], in0=es[0:1, :], scalar1=ds[0:1, 0:1]
                )
                pb = p3p.tile([P, 2], F32, tag="b")
                nc.tensor.matmul(
                    out=pb[:], lhsT=ones_sb[:], rhs=at[0:1, :],
                    start=True, stop=True,
                )
                ab = p3s.tile([P, 2], F32, tag="ab")
                nc.vector.tensor_copy(out=ab[:], in_=pb[:])
                for t in range(NT):
                    oe = p3s.tile([P, P], F32, tag="oe")
                    om = p3s.tile([P, P], F32, tag="om")
                    nc.sync.dma_start(out=oe[:], in_=oemT[:, t * P:(t + 1) * P])
                    nc.sync.dma_start(out=om[:], in_=ommT[:, t * P:(t + 1) * P])
                    comb = p3s.tile([P, P], F32, tag="comb")
                    nc.vector.tensor_scalar_mul(
                        out=comb[:], in0=oe[:], scalar1=ab[:, 0:1]
                    )
                    nc.vector.scalar_tensor_tensor(
                        out=comb[:],
                        in0=om[:],
                        scalar=ab[:, 1:2],
                        in1=comb[:],
                        op0=mybir.AluOpType.mult,
                        op1=mybir.AluOpType.add,
                    )
                    py_ = p3p.tile([P, OUT], F32, tag="y")
                    nc.tensor.matmul(
                        out=py_[:], lhsT=comb[:], rhs=wlT_sb[:],
                        start=True, stop=True,
                    )
                    y_sb = p3s.tile([P, OUT], F32, tag="ysb")
                    nc.vector.tensor_add(out=y_sb[:], in0=py_[:], in1=blb_sb[:])
                    nc.sync.dma_start(out=y[t * P:(t + 1) * P, :], in_=y_sb[:])

    nc.compile()
    return nc


_CACHE = {}


def prep_all(inputs):
    host = build_host_tensors(inputs)
    s_em, a_em, d_em, ch_em = bucket_edges(inputs["src_em"], inputs["dst_em"], 0)
    s_mm, a_mm, d_mm, ch_mm = bucket_edges(inputs["src_mm"], inputs["dst_mm"], 1)
    in_maps = []
    for c in range(NCORES):
        m = dict(host)
        m["esrc"] = np.ascontiguousarray(
            np.concatenate([s_em[c], s_mm[c]], axis=2))
        m["ead"] = np.ascontiguousarray(
            np.concatenate([a_em[c], a_mm[c]], axis=2))
        m["edl"] = np.ascontiguousarray(
            np.concatenate([d_em[c], d_mm[c]], axis=2))
        in_maps.append(m)
    return in_maps, ch_em, ch_mm


def kernel(**inputs):
    in_maps, ch_em, ch_mm = prep_all(inputs)
    key = (ch_em, ch_mm)
    if key not in _CACHE:
        _CACHE[key] = build_program(ch_em, ch_mm)
    nc = _CACHE[key]
    res = run_bass_kernel_spmd(nc, in_maps, core_ids=list(range(NCORES)))
    out = np.empty((N_MAT, OUT), np.float32)
    for c in range(NCORES):
        out[c * ND:(c + 1) * ND] = res.results[c]["y"][:ND]
    return out

